# revision 32
# baseline (speedup 1.0000x reference)
"""Bass/Trainium2 kernel for nn_Differential_Attention_60825326846200.

Mathematical reduction of the reference:
  scores[b,h,i,j] = (sum_d q[b,h,i,d] - k[b,h,i,d]) / sqrt(DH) + mask[b,i]
is constant over the key index j, so the softmax over j is exactly the
uniform distribution (1/S) regardless of q, k, and the mask.  Hence
  ctx[b,h,i,:] = mean_j v[b,h,j,:]          (independent of i)
  out[b,i,:]   = (mean_j hidden_b[b,j,:]) @ Wv.T + bv   for every i.
The q/k projections and the attention mask cancel exactly.

Distribution across the 8 NeuronCores — two small collective-free SPMD
launches (a cross-core AllReduce costs 40-55us here in barrier+mesh
latency; two extra launch fixed-overheads are cheaper).  Shards are laid
out on the host in transposed, partition-tiled form (pure permutation,
no host arithmetic) so each launch needs no on-device transposes:

  Launch 1 (mean, sequence-sharded): core c gets its [B, S/8, HID]
  slice of hidden_b as [128(p), KC(kc), B, S/8] (k = kc*128+p on the
  partition axis) and reduces the trailing sequence axis with two DVE
  reduce_sum instructions -> 8KB partial-sum output "part" [128, KC*B].

  Host glue (pure data movement): stack the 8 partial tensors along a
  trailing axis; replicate to all cores.

  Launch 2 (projection, feature-sharded): each core sums the 8 partials
  (one DVE reduce), scales by 1/S, computes
  out_row[b, o] = sum_k hbar.T[k,b] * Wv.T[k,o] with 16 M=1 tensor-
  engine matmuls against its host-pre-transposed Wv shard, adds the
  bias, broadcasts both rows across the 128-partition axis with one
  rank-1 matmul, and DMA-writes its [B, S, 128] output slice with
  broadcast (step-0) source access patterns.

Host does data movement only: slicing/permutation/concatenation.
"""

import numpy as np

import concourse.bacc as bacc
import concourse.mybir as mybir
import concourse.tile as tile
from concourse.bass_utils import run_bass_kernel_spmd

N_CORES = 8
B, S, HID = 2, 2048, 1024
S_LOC = S // N_CORES  # 256 sequence positions reduced per core
O_LOC = HID // N_CORES  # 128 output features produced per core
KC = HID // 128  # 8 contraction chunks of 128
F32 = mybir.dt.float32

_compiled = None


def _new_nc():
    return bacc.Bacc(
        "TRN2",
        target_bir_lowering=False,
        debug=False,
        enable_asserts=False,
        num_devices=N_CORES,
    )


def _build_mean():
    """Launch 1: partial column-sum of this core's hidden_b slice.
    Input "hbt" [128, KC, B, S_LOC]: hbt[p, kc, b, s] = hb[b, s, kc*128+p].
    Output "part" [128, KC*B] with column kc*B + b (raw sums, unscaled)."""
    nc = _new_nc()
    hbt = nc.dram_tensor("hbt", [128, KC, B, S_LOC], F32, kind="ExternalInput").ap()
    part = nc.dram_tensor("part", [128, KC * B], F32, kind="ExternalOutput").ap()

    # Raw bass (no TileContext): hand-placed semaphores avoid Tile's
    # kernel-tail drain + double all-engine barrier + bulk sem recycling.
    # Each DMA gets its own semaphore (the race analyzer requires
    # deterministic per-sem trajectories across concurrently-completing
    # split sub-DMAs).
    nch = 8  # DMA/reduce pipeline chunks (one kc each), 2 HWDGE rings
    chunk = B * S_LOC  # free elements per chunk
    # Bacc's lowering splits each 512KB HWDGE DMA into 3 sub-DMAs, each
    # incrementing its semaphore by 16 (empirical; the CoreSim race
    # detector rejects any wait that disagrees, so this is checked on
    # every build). The tiny 8KB result DMAs stay unsplit.
    CHUNK_INC = 16
    from contextlib import ExitStack

    with ExitStack() as ctx:
        tsb = ctx.enter_context(
            nc.sbuf_tensor("tsb", [128, KC * B * S_LOC], F32)
        )
        part_sb = ctx.enter_context(nc.sbuf_tensor("part_sb", [128, KC * B], F32))
        csem = [
            ctx.enter_context(nc.semaphore(f"c{h}")) for h in range(nch)
        ]
        out0 = ctx.enter_context(nc.semaphore("out0"))
        out1 = ctx.enter_context(nc.semaphore("out1"))
        red = ctx.enter_context(nc.semaphore("red"))
        block = ctx.enter_context(nc.Block())

        def chunk_view(h):
            return tsb[:, h * chunk : (h + 1) * chunk].rearrange(
                "p (b s) -> p b s", b=B
            )

        @block.sync
        def _(sync):
            for h in (0, 2, 4, 6):
                sync.dma_start(
                    tsb[:, h * chunk : (h + 1) * chunk],
                    hbt[:, h].rearrange("p b s -> p (b s)"),
                ).then_inc(csem[h], 16)
            sync.wait_ge(red, nch)
            sync.dma_start(
                part[:, (nch - 1) * B :], part_sb[:, (nch - 1) * B :]
            ).then_inc(out1, 16)

        @block.scalar
        def _(scalar):
            for h in (1, 3, 5, 7):
                scalar.dma_start(
                    tsb[:, h * chunk : (h + 1) * chunk],
                    hbt[:, h].rearrange("p b s -> p (b s)"),
                ).then_inc(csem[h], 16)
            # overlap most of the tiny result write with the last reduce
            scalar.wait_ge(red, nch - 1)
            scalar.dma_start(
                part[:, : (nch - 1) * B], part_sb[:, : (nch - 1) * B]
            ).then_inc(out0, 16)

        @block.vector
        def _(vector):
            for h in range(nch):
                vector.wait_ge(csem[h], CHUNK_INC)
                vector.reduce_sum(
                    part_sb[:, h * B : (h + 1) * B],
                    chunk_view(h),
                    axis=mybir.AxisListType.X,
                ).then_inc(red, 1)

        @block.gpsimd
        def _(g):
            # all sems at final values => every DMA/compute increment has
            # landed before the Block-exit engine barrier runs
            for h in range(nch):
                g.wait_ge(csem[h], CHUNK_INC)
            g.wait_ge(red, nch)
            g.wait_ge(out0, 16)
            g.wait_ge(out1, 16)

        # Block exit emits an all-engine barrier; after it, reset the
        # semaphores so the NEFF is safely re-executable
        block.close_and_clear = True  # marker (no-op)
        nums = sorted(s.num for s in csem + [out0, out1, red])
        assert nums == list(range(nums[0], nums[0] + len(nums)))
        sem_rng = range(nums[0], nums[-1] + 1)

    nc.gpsimd.dma_reset(sem_rng)
    nc.gpsimd.sem_clear(sem_rng)
    nc.compile()
    return nc


def _build_proj():
    """Launch 2: sum the 8 partials, scale by 1/S, project through this
    core's (host-pre-transposed) Wv rows, add bias, broadcast over the
    sequence axis, write the [B, S, O_LOC] output slice.
    Input "wvt" [128, KC, O_LOC]: wvt[p, kc, o] = Wv[c*128+o, kc*128+p]."""
    nc = _new_nc()
    parts = nc.dram_tensor(
        "parts", [128, KC * B, N_CORES], F32, kind="ExternalInput"
    ).ap()
    wvt = nc.dram_tensor("wvt", [128, KC, O_LOC], F32, kind="ExternalInput").ap()
    bv = nc.dram_tensor("bv", [1, O_LOC], F32, kind="ExternalInput").ap()
    out = nc.dram_tensor("out", [B, S, O_LOC], F32, kind="ExternalOutput").ap()

    with tile.TileContext(nc) as tc:
        with (
            tc.tile_pool(name="big", bufs=1) as big,
            tc.tile_pool(name="small", bufs=1) as small,
            tc.tile_pool(name="psum", bufs=1, space="PSUM") as psum,
        ):
            ones1 = small.tile([1, 128], F32)
            nc.vector.memset(ones1[:], 1.0)

            # parts gates everything downstream: load it first on sync;
            # wvt split across the other two DMA paths
            parts_sb = small.tile([128, KC * B * N_CORES], F32)
            nc.sync.dma_start(parts_sb[:], parts[:])
            bv_sb = small.tile([1, O_LOC], F32)
            nc.sync.dma_start(bv_sb[:], bv[:])

            wvT = big.tile([128, KC * O_LOC], F32)
            for h, eng in enumerate((nc.scalar, nc.gpsimd)):
                hk = KC // 2
                eng.dma_start(
                    wvT[:, h * hk * O_LOC : (h + 1) * hk * O_LOC].rearrange(
                        "p (kc o) -> p kc o", kc=hk
                    ),
                    wvt[:, h * hk : (h + 1) * hk],
                )
            hbT = small.tile([128, KC * B], F32)
            nc.vector.reduce_sum(
                hbT[:],
                parts_sb[:].rearrange("p (c n) -> p c n", n=N_CORES),
                axis=mybir.AxisListType.X,
            )
            nc.vector.tensor_scalar_mul(hbT[:], hbT[:], 1.0 / S)

            # out_row[b, o] = sum_k hbar.T[k, b] * Wv.T[k, o]  (M=1 chains,
            # fused weight loads); bias added on DVE while moving the row
            # to SBUF; a rank-1 matmul broadcasts it across 128 partitions.
            # Separate PSUM tiles per batch -> separate banks, so batch 1's
            # matmuls overlap batch 0's copy/DMA instead of serializing.
            tsplit = [0, 5, 10, S // 128]  # output thirds per engine
            engs = (nc.sync, nc.scalar, nc.gpsimd)
            for b in range(B):
                psum_row = psum.tile(
                    [1, O_LOC], F32, name=f"prow{b}", tag=f"prow{b}"
                )
                for kc in range(KC):
                    nc.tensor.matmul(
                        psum_row[:],
                        lhsT=hbT[:, kc * B + b : kc * B + b + 1],
                        rhs=wvT[:, kc * O_LOC : (kc + 1) * O_LOC],
                        start=(kc == 0),
                        stop=(kc == KC - 1),
                    )
                row_f = small.tile([1, O_LOC], F32, name=f"rowf{b}")
                nc.vector.tensor_add(row_f[:], psum_row[:], bv_sb[:])

                pbc = psum.tile([128, O_LOC], F32, name=f"pbc{b}", tag=f"pbc{b}")
                nc.tensor.matmul(
                    pbc[:], lhsT=ones1[:], rhs=row_f[:], start=True, stop=True
                )
                bcb = big.tile([128, O_LOC], F32, name=f"bc{b}", tag=f"bc{b}")
                nc.vector.tensor_copy(bcb[:], pbc[:])

                # write this batch's [S, O_LOC] slice reusing the 128-row
                # tile 16x (step-0 source AP), split across 3 DMA paths
                dst = out[b].rearrange("(t p) o -> p t o", p=128)
                for eng, lo, hi in zip(engs, tsplit, tsplit[1:]):
                    src = bcb[:].unsqueeze(1).broadcast_to([128, hi - lo, O_LOC])
                    eng.dma_start(dst[:, lo:hi], src)
    nc.compile()
    return nc


def get_ncs():
    global _compiled
    if _compiled is None:
        _compiled = (_build_mean(), _build_proj())
    return _compiled


def make_mean_in_maps(inputs):
    hb = np.asarray(inputs["hidden_states_b"], dtype=np.float32)
    # [B, S, HID] -> per core [128, KC, B, S_LOC] (pure permutation)
    maps = []
    for c in range(N_CORES):
        sl = hb[:, c * S_LOC : (c + 1) * S_LOC, :]  # [B, S_LOC, HID]
        t = sl.reshape(B, S_LOC, KC, 128).transpose(3, 2, 0, 1)
        maps.append({"hbt": np.ascontiguousarray(t)})
    return maps


def make_proj_in_maps(inputs, part_results):
    Wv = np.asarray(inputs["Wv"], dtype=np.float32)
    bv = np.asarray(inputs["bv"], dtype=np.float32)
    parts = np.ascontiguousarray(
        np.stack([part_results[c]["part"] for c in range(N_CORES)], axis=-1)
    )
    maps = []
    for c in range(N_CORES):
        w = Wv[c * O_LOC : (c + 1) * O_LOC, :]  # [O_LOC, HID]
        wt = w.reshape(O_LOC, KC, 128).transpose(2, 1, 0)  # [128, KC, O_LOC]
        maps.append(
            {
                "parts": parts,
                "wvt": np.ascontiguousarray(wt),
                "bv": np.ascontiguousarray(
                    bv[c * O_LOC : (c + 1) * O_LOC].reshape(1, O_LOC)
                ),
            }
        )
    return maps


def gather_out(results):
    return np.concatenate([results[c]["out"] for c in range(N_CORES)], axis=2)


def kernel(**inputs) -> np.ndarray:
    nc_mean, nc_proj = get_ncs()
    cores = list(range(N_CORES))
    res1 = run_bass_kernel_spmd(nc_mean, make_mean_in_maps(inputs), cores)
    res2 = run_bass_kernel_spmd(nc_proj, make_proj_in_maps(inputs, res1.results), cores)
    return gather_out(res2.results)
